# revision 2
# baseline (speedup 1.0000x reference)
"""Trainium2 Bass kernel for nn_AtomAttention (B=2, N=2048, D=256, C=4, H=4).

Key algebraic property of the reference:

    weighted = einsum('bqkh,bvdh->bqdh', att, v)

has NO shared summation index between `att` and `v` (`k` and `v` are summed
independently), so it factorizes into

    weighted[b,q,d,h] = (sum_k att[b,q,k,h]) * (sum_v v[b,v,d,h])

and since `att` is a softmax over axis k, the first factor is exactly 1 for
every (b,q,h) — regardless of the attention scores, bias, mask or scaling.
Therefore the whole network reduces exactly (not approximately) to

    vsum[b,:]  = sum_n (atom_embed[b] @ Wv)[n, :]          # (B, D*H)
    gate       = sigmoid(atom_embed @ Wg + bg)             # (B, N, D*H)
    out        = (gate * vsum[:,None,:]) @ Wo + bo         # (B, N, D)

(verified: f64 full-reference vs f64 shortcut agree to ~7e-15 rel, including
with non-trivial masks). molecular_matrix / Wq / Wk / W_bias / layernorm
params / embedding_mask cancel out of the forward value entirely.

Sharding: 8 cores, data-parallel: core c handles batch b=c//4, query rows
[s*512, (s+1)*512) with s=c%4. Each core receives the full (transposed) embed
matrix of its batch — columns rolled so its own 512 rows come first — so it
can compute the batch-wide column-sum `vsum` locally (no collectives), plus
replicated weights.

Per-core device pipeline (all f32):
  et   (256,2048) = E[b]^T (rolled)          -> SBUF (2 tiles of 128p)
  esum (256,1)    = row-sum of et (free-axis reduce on DVE)
  gateT tiles (128,512) = sigmoid(Wg^T @ E_own^T + bg)   (PE + ACT)
  vsumT (128,8)   = Wv^T-chunks @ esum                    (PE)
  Wo'  = Wo rows scaled by vsum (per-partition tensor_scalar on DVE)
  out  (512,256)  = gateT^T @ Wo' + bo (bias via ones^T@bo PSUM-init matmul)
"""

import numpy as np

import concourse.bacc as bacc
import concourse.tile as tile
from concourse import mybir
from concourse.bass_utils import run_bass_kernel_spmd

B, N, D, H = 2, 2048, 256, 4
DH = D * H            # 1024
NCORES = 8
CPB = NCORES // B     # cores per batch = 4
ROWS = N // CPB       # rows per core = 512
P = 128
KC = D // P           # contraction chunks over embed dim = 2
TT = DH // P          # chunks over the fused (d,h) axis = 8
MC = ROWS // P        # output row chunks per core = 4
F32 = mybir.dt.float32


def build_nc():
    nc = bacc.Bacc("TRN2", target_bir_lowering=False, debug=False,
                   num_devices=NCORES)

    et = nc.dram_tensor("et", [D, N], F32, kind="ExternalInput")
    wg = nc.dram_tensor("wg", [D, DH], F32, kind="ExternalInput")
    wv = nc.dram_tensor("wv", [D, DH], F32, kind="ExternalInput")
    wo = nc.dram_tensor("wo", [DH, D], F32, kind="ExternalInput")
    bgt = nc.dram_tensor("bgt", [P, TT], F32, kind="ExternalInput")
    bo = nc.dram_tensor("bo", [1, D], F32, kind="ExternalInput")
    ones = nc.dram_tensor("ones", [1, P], F32, kind="ExternalInput")
    out = nc.dram_tensor("out", [ROWS, D], F32, kind="ExternalOutput")

    with tile.TileContext(nc) as tc:
        with (
            tc.tile_pool(name="sb", bufs=1) as sb,
            tc.tile_pool(name="osb", bufs=2) as osb,
            tc.tile_pool(name="ps_v", bufs=1, space="PSUM") as ps_v,
            tc.tile_pool(name="ps_g", bufs=2, space="PSUM") as ps_g,
            tc.tile_pool(name="ps_o", bufs=2, space="PSUM") as ps_o,
        ):
            et_t = [sb.tile([P, N], F32, name=f"et{c}", tag=f"et{c}") for c in range(KC)]
            wg_t = [sb.tile([P, DH], F32, name=f"wg{c}", tag=f"wg{c}") for c in range(KC)]
            wv_t = [sb.tile([P, DH], F32, name=f"wv{c}", tag=f"wv{c}") for c in range(KC)]
            wo_t = [sb.tile([P, D], F32, name=f"wo{t}", tag=f"wo{t}") for t in range(TT)]
            wos_t = [sb.tile([P, D], F32, name=f"wos{t}", tag=f"wos{t}") for t in range(TT)]
            gt_t = [sb.tile([P, ROWS], F32, name=f"gt{t}", tag=f"gt{t}") for t in range(TT)]
            bgt_t = sb.tile([P, TT], F32, tag="bgt")
            bo_t = sb.tile([1, D], F32, tag="bo")
            one_t = sb.tile([1, P], F32, tag="ones")
            es_t = [sb.tile([P, 1], F32, name=f"es{c}", tag=f"es{c}") for c in range(KC)]
            vs_sb = sb.tile([P, TT], F32, tag="vs")

            for c in range(KC):
                nc.sync.dma_start(et_t[c][:], et[c * P:(c + 1) * P, :])
            for c in range(KC):
                nc.sync.dma_start(wg_t[c][:], wg[c * P:(c + 1) * P, :])
            for c in range(KC):
                nc.sync.dma_start(wv_t[c][:], wv[c * P:(c + 1) * P, :])
            for t in range(TT):
                nc.sync.dma_start(wo_t[t][:], wo[t * P:(t + 1) * P, :])
            nc.sync.dma_start(bgt_t[:], bgt[:])
            nc.sync.dma_start(bo_t[:], bo[:])
            nc.sync.dma_start(one_t[:], ones[:])

            # esum over atoms (free axis) for each 128-row chunk of E^T
            for c in range(KC):
                nc.vector.reduce_sum(es_t[c][:], et_t[c][:],
                                     axis=mybir.AxisListType.X)

            # gate^T tiles: sigmoid(Wg^T @ E_own^T + bg)
            for t in range(TT):
                g_ps = ps_g.tile([P, ROWS], F32)
                for c in range(KC):
                    nc.tensor.matmul(g_ps[:],
                                     wg_t[c][:, t * P:(t + 1) * P],
                                     et_t[c][:, 0:ROWS],
                                     start=(c == 0), stop=(c == KC - 1))
                nc.scalar.activation(gt_t[t][:], g_ps[:],
                                     mybir.ActivationFunctionType.Sigmoid,
                                     bias=bgt_t[:, t:t + 1])

            # vsum^T[p, t] = sum_d esum[d] * Wv[d, t*128+p]
            vs_ps = ps_v.tile([P, TT], F32)
            for t in range(TT):
                for c in range(KC):
                    nc.tensor.matmul(vs_ps[:, t:t + 1],
                                     wv_t[c][:, t * P:(t + 1) * P],
                                     es_t[c][:],
                                     start=(c == 0), stop=(c == KC - 1))
            nc.vector.tensor_copy(vs_sb[:], vs_ps[:])

            # Wo' = rows of Wo scaled by vsum
            for t in range(TT):
                nc.vector.tensor_scalar_mul(wos_t[t][:], wo_t[t][:],
                                            vs_sb[:, t:t + 1])

            # out rows: PSUM init with broadcast bo (ones^T @ bo), then
            # accumulate gate^T-chunk^T @ Wo'
            for m in range(MC):
                o_ps = ps_o.tile([P, D], F32)
                nc.tensor.matmul(o_ps[:], one_t[:], bo_t[:],
                                 start=True, stop=False)
                for t in range(TT):
                    nc.tensor.matmul(o_ps[:],
                                     gt_t[t][:, m * P:(m + 1) * P],
                                     wos_t[t][:],
                                     start=False, stop=(t == TT - 1))
                o_sb = osb.tile([P, D], F32, name="o", tag="o")
                nc.scalar.copy(o_sb[:], o_ps[:])
                nc.sync.dma_start(out[m * P:(m + 1) * P, :], o_sb[:])

    nc.compile()
    return nc


_NC = None


def _get_nc():
    global _NC
    if _NC is None:
        _NC = build_nc()
    return _NC


def _make_in_maps(inputs):
    E = np.asarray(inputs["atom_embed"], dtype=np.float32)
    Wg = np.ascontiguousarray(np.asarray(inputs["Wg"], dtype=np.float32))
    Wv = np.ascontiguousarray(np.asarray(inputs["Wv"], dtype=np.float32))
    Wo = np.ascontiguousarray(np.asarray(inputs["Wo"], dtype=np.float32))
    bg = np.asarray(inputs["bg"], dtype=np.float32)
    bo = np.asarray(inputs["bo"], dtype=np.float32)

    bgt = np.ascontiguousarray(bg.reshape(TT, P).T)
    bo2 = np.ascontiguousarray(bo.reshape(1, D))
    ones = np.ones((1, P), dtype=np.float32)

    in_maps = []
    for c in range(NCORES):
        b, s = divmod(c, CPB)
        ET = E[b].T  # (D, N)
        rolled = np.concatenate([ET[:, s * ROWS:], ET[:, :s * ROWS]], axis=1)
        in_maps.append({
            "et": np.ascontiguousarray(rolled),
            "wg": Wg, "wv": Wv, "wo": Wo,
            "bgt": bgt, "bo": bo2, "ones": ones,
        })
    return in_maps


def _run(inputs, trace=False):
    """Run on 8 NeuronCores; returns (full_output, BassKernelResults)."""
    in_maps = _make_in_maps(inputs)
    res = run_bass_kernel_spmd(_get_nc(), in_maps, list(range(NCORES)),
                               trace=trace)
    out = np.empty((B, N, D), dtype=np.float32)
    for c in range(NCORES):
        b, s = divmod(c, CPB)
        out[b, s * ROWS:(s + 1) * ROWS, :] = res.results[c]["out"]
    return out, res


def kernel(**inputs) -> np.ndarray:
    out, _ = _run(inputs, trace=False)
    return out
